# revision 1
# baseline (speedup 1.0000x reference)
"""Trainium2 Bass kernel for conv-qkv rank-1 attention.

out = gamma * q * sum(k*v) + x, where q,k,v are per-time-slice 3x3 convs
(C=64 -> C=64) of x [B=8, C=64, T=16, W=64, H=64].

Sharding: data-parallel over B across 8 cores (1 example/core), conv
weights replicated. No cross-core communication.

Per-core schedule: T slices processed in pairs; slice t lives on SBUF
partitions 0-63, slice t+1 on partitions 64-127, giving two concurrent
PE row-group chains (K=64 each). Each 3x3 conv = 9 shifted matmul taps
(+1 bias tap with an all-ones rhs) accumulated in PSUM. Stationary
[Wq|Wk] (M=128) produces q,k in one bank; Wv (M=64) is column-placed so
k and v land on the same partitions for the fused DVE k*v+reduce.
Matmuls run in float32r (FP22 truncation on read, 1 cycle/row).
"""

import numpy as np

import concourse.bacc as bacc
import concourse.bass as bass
import concourse.mybir as mybir
import concourse.tile as tile
from concourse import bass_utils

F32 = mybir.dt.float32
F32R = mybir.dt.float32r
ALU = mybir.AluOpType

B, C, T, W, H = 8, 64, 16, 64, 64
WP, HP = W + 2, H + 2          # padded slice dims
import os
NPAIR = int(os.environ.get("BASS_NPAIR", T // 2))  # slice pairs per core
RB = 8                         # W-rows per pixel block
NBLK = W // RB                 # pixel blocks per slice
BN = RB * H                    # moving free dim per matmul (512)
NTAP = 10                      # 9 conv taps + 1 bias tap


def _round22(a: np.ndarray) -> np.ndarray:
    """Round fp32 to 11 mantissa bits so the PE's FP22 read-truncation is
    exact (unbiased quantization instead of truncation)."""
    u = np.ascontiguousarray(a, np.float32).view(np.uint32).astype(np.uint64)
    u = ((u + 0x800) & 0xFFFFF000).astype(np.uint32)
    return u.view(np.float32)


def _pack_weights(wq, wk, wv, bq, bk, bv):
    """Pack stationary operands.

    wqk [128, 10, 128]: partitions 0-63 = chain-low taps ([Wq | Wk] so q
    lands on psum partitions 0-63, matching x_t's partitions), partitions
    64-127 = chain-high taps ([Wk | Wq], q on partitions 64-127). Tap 9 is
    the bias tap (row 0 = biases, used with an all-ones rhs).
    wv2 [128, 10, 64]: Wv taps for both chains (same values).
    """
    def taps(w):  # [O, I, 1, 3, 3] -> [I, 9, O]
        return np.ascontiguousarray(
            w.reshape(C, C, 9).transpose(1, 2, 0), np.float32)

    wq_t, wk_t, wv_t = taps(wq), taps(wk), taps(wv)
    # [Wk | Wq] for both chains: k lands on psum partitions 0-63 (the
    # custom DVE reduce op requires base partition 0), q on 64-127
    wqk = np.zeros((128, NTAP, 128), np.float32)
    wqk[0:64, 0:9, 0:64] = wk_t
    wqk[0:64, 0:9, 64:128] = wq_t
    wqk[64:128, 0:9, 0:64] = wk_t
    wqk[64:128, 0:9, 64:128] = wq_t
    wqk[0, 9, 0:64] = bk
    wqk[0, 9, 64:128] = bq
    wqk[64, 9, 0:64] = bk
    wqk[64, 9, 64:128] = bq

    # v stationary is [Wv | Wv] (M=128): the duplicated column half costs
    # nothing (M=64 would leave the array half idle) and lets every matmul
    # use column position 0, which fp32r codegen requires
    wv2 = np.zeros((128, NTAP, 128), np.float32)
    wv2[0:64, 0:9, 0:64] = wv_t
    wv2[0:64, 0:9, 64:128] = wv_t
    wv2[64:128, 0:9, 0:64] = wv_t
    wv2[64:128, 0:9, 64:128] = wv_t
    wv2[0, 9, 0:64] = bv
    wv2[0, 9, 64:128] = bv
    wv2[64, 9, 0:64] = bv
    wv2[64, 9, 64:128] = bv
    return _round22(wqk), _round22(wv2)


def _emit(nc, tc, x_d, wqk_d, wv_d, gam_d, ones_d, zer_d, out_d, ctx):
    const = ctx.enter_context(tc.tile_pool(name="const", bufs=1))
    state = ctx.enter_context(tc.tile_pool(name="state", bufs=1))
    psum = ctx.enter_context(
        tc.tile_pool(name="psum", bufs=2, space=bass.MemorySpace.PSUM))
    vpool = ctx.enter_context(tc.tile_pool(name="vpool", bufs=2))

    wqk_t = const.tile([128, NTAP, 128], F32R, tag="wqk")
    wv_t = const.tile([128, NTAP, 128], F32R, tag="wv")
    gam_t = const.tile([128, 1], F32, tag="gam")
    ones_t = const.tile([128, BN], F32R, tag="ones")

    nc.sync.dma_start(wqk_t[:], wqk_d[:])
    nc.sync.dma_start(wv_t[:], wv_d[:])
    nc.sync.dma_start(gam_t[:], gam_d[:])
    nc.sync.dma_start(ones_t[:], ones_d[:])

    xp = [state.tile([128, WP, HP], F32R, tag=f"xp{i}", name=f"xp{i}") for i in range(3)]
    qs = [state.tile([128, W * H], F32, tag=f"qs{i}", name=f"qs{i}") for i in range(2)]
    ot = [state.tile([128, W * H], F32, tag=f"ot{i}", name=f"ot{i}") for i in range(2)]
    scr = state.tile([128, BN], F32, tag="scr")
    sparts = [state.tile([64, 2, NBLK], F32, tag=f"sp{i}", name=f"sp{i}") for i in range(2)]
    sgam = [state.tile([64, 2], F32, tag=f"sg{i}", name=f"sg{i}") for i in range(2)]
    sfin = [state.tile([128, 1], F32, tag=f"sf{i}", name=f"sf{i}") for i in range(2)]

    # zero the padding ring of both x buffers once (gpsimd memset does not
    # take f32r, so DMA from a host-provided zero vector); interior DMAs
    # never touch the ring
    for t_ in xp:
        nc.sync.dma_start(t_[:, 0, :], zer_d[:, :])
        nc.sync.dma_start(t_[:, WP - 1, :], zer_d[:, :])
        nc.sync.dma_start(t_[:, :, 0], zer_d[:, 0:WP])
        nc.sync.dma_start(t_[:, :, HP - 1], zer_d[:, 0:WP])

    def load_pair(p):
        t_ = xp[p % 3]
        nc.sync.dma_start(t_[0:64, 1:1 + W, 1:1 + H], x_d[:, 2 * p])
        nc.sync.dma_start(t_[64:128, 1:1 + W, 1:1 + H], x_d[:, 2 * p + 1])

    load_pair(0)
    if NPAIR > 1:
        load_pair(1)

    for p in range(NPAIR):
        pb = p % 2
        xp_, qs_, ot_ = xp[p % 3], qs[pb], ot[pb]

        # prefetch two pairs ahead; emitted before this pair's s-swap DMA
        # so the serial sync queue never holds the x-load behind it
        if p + 2 < NPAIR:
            load_pair(p + 2)

        for j in range(NBLK):
            qk_lo = psum.tile([128, BN], F32, tag="qk_lo")
            qk_hi = psum.tile([128, BN], F32, tag="qk_hi")
            v_lo = psum.tile([128, BN], F32, tag="v_lo", name="v_lo")
            v_hi = psum.tile([128, BN], F32, tag="v_hi", name="v_hi")

            def rhs(half, tap):
                if tap == 9:
                    return ones_t[64 * half:64 * half + 64, :]
                dy, dx = tap // 3, tap % 3
                r0 = j * RB + dy
                return xp_[64 * half:64 * half + 64,
                           r0:r0 + RB, dx:dx + H]

            for tap in range(NTAP):
                st, sp = tap == 0, tap == NTAP - 1
                nc.tensor.matmul(
                    qk_lo[:, :],
                    wqk_t[0:64, tap, :],
                    rhs(0, tap), start=st, stop=sp)
                nc.tensor.matmul(
                    qk_hi[:, :],
                    wqk_t[64:128, tap, :],
                    rhs(1, tap), start=st, stop=sp)
            for tap in range(NTAP):
                st, sp = tap == 0, tap == NTAP - 1
                nc.tensor.matmul(
                    v_lo[:, :],
                    wv_t[0:64, tap, :],
                    rhs(0, tap), start=st, stop=sp)
                nc.tensor.matmul(
                    v_hi[:, :],
                    wv_t[64:128, tap, :],
                    rhs(1, tap), start=st, stop=sp)

            # evacuate q and v on ScalarE (DVE may read only one PSUM
            # operand, so v must reach SBUF before the fused k*v reduce).
            # q_t moves partitions 64-127 -> 0-63 to line up with x_t.
            if os.environ.get("BASS_QCROSS", "1") == "1":
                nc.scalar.copy(qs_[0:64, j * BN:(j + 1) * BN], qk_lo[64:128, :])
            else:
                nc.scalar.copy(qs_[0:64, j * BN:(j + 1) * BN], qk_lo[0:64, :])
            nc.scalar.copy(qs_[64:128, j * BN:(j + 1) * BN], qk_hi[64:128, :])
            vsb_lo = vpool.tile([64, BN], F32, tag="vsb_lo", name="vsb_lo")
            vsb_hi = vpool.tile([64, BN], F32, tag="vsb_hi", name="vsb_hi")
            nc.scalar.copy(vsb_lo[:, :], v_lo[0:64, :])
            nc.scalar.copy(vsb_hi[:, :], v_hi[0:64, :])

            # fused k*v multiply + pixel-sum (k from PSUM at base partition
            # 0 -- the custom DVE op requires it; v from SBUF)
            if os.environ.get("BASS_TTR", "1") == "1":
                # native TensorScalarPtr with accumulate: one DVE pass does
                # k*v and the pixel-sum
                nc.vector.scalar_tensor_tensor(
                    out=scr[0:64, :], in0=qk_lo[0:64, :], scalar=1.0,
                    in1=vsb_lo[:, :], op0=ALU.mult, op1=ALU.mult,
                    accum_out=sparts[pb][:, 0, j:j + 1])
                nc.vector.scalar_tensor_tensor(
                    out=scr[0:64, :], in0=qk_hi[0:64, :], scalar=1.0,
                    in1=vsb_hi[:, :], op0=ALU.mult, op1=ALU.mult,
                    accum_out=sparts[pb][:, 1, j:j + 1])
            else:
                nc.vector.tensor_tensor(
                    out=scr[0:64, :], in0=qk_lo[0:64, :], in1=vsb_lo[:, :],
                    op=ALU.mult)
                nc.vector.reduce_sum(sparts[pb][:, 0, j:j + 1], scr[0:64, :],
                                     axis=mybir.AxisListType.X)
                nc.vector.tensor_tensor(
                    out=scr[0:64, :], in0=qk_hi[0:64, :], in1=vsb_hi[:, :],
                    op=ALU.mult)
                nc.vector.reduce_sum(sparts[pb][:, 1, j:j + 1], scr[0:64, :],
                                     axis=mybir.AxisListType.X)

        nc.vector.reduce_sum(sgam[pb][:, :], sparts[pb][:, :, :],
                             axis=mybir.AxisListType.X)
        nc.vector.tensor_scalar_mul(sgam[pb][:, :], sgam[pb][:, :],
                                    gam_t[0:64, 0:1])
        # s_{t+1} is accumulated on partitions 0-63 but q_{t+1}/x_{t+1}
        # live on 64-127: move it with a tiny sbuf->sbuf DMA
        if os.environ.get("BASS_SWAPDMA", "1") == "1":
            nc.sync.dma_start(sfin[pb][64:128, :], sgam[pb][:, 1:2])
        else:
            nc.vector.tensor_copy(sfin[pb][0:64, :], sgam[pb][:, 1:2])

        for j in range(NBLK):
            # out = (q * (gamma*s)) + x, fused
            nc.vector.scalar_tensor_tensor(
                out=ot_[0:64, j * BN:(j + 1) * BN],
                in0=qs_[0:64, j * BN:(j + 1) * BN],
                scalar=sgam[pb][:, 0:1],
                in1=xp_[0:64, 1 + j * RB:1 + (j + 1) * RB, 1:1 + H].bitcast(F32),
                op0=ALU.mult, op1=ALU.add)
            nc.vector.scalar_tensor_tensor(
                out=ot_[64:128, j * BN:(j + 1) * BN],
                in0=qs_[64:128, j * BN:(j + 1) * BN],
                scalar=sfin[pb][64:128, 0:1],
                in1=xp_[64:128, 1 + j * RB:1 + (j + 1) * RB, 1:1 + H].bitcast(F32),
                op0=ALU.mult, op1=ALU.add)

        nc.gpsimd.dma_start(out_d[:, 2 * p], ot_[0:64, :])
        nc.gpsimd.dma_start(out_d[:, 2 * p + 1], ot_[64:128, :])


_ONES = np.ones((128, BN), np.float32)
_ZER = np.zeros((128, HP), np.float32)

_CACHE = {}


def _build():
    if "nc" in _CACHE:
        return _CACHE["nc"]
    nc = bacc.Bacc("TRN2", target_bir_lowering=False, debug=False,
                   enable_asserts=False, num_devices=8)
    x_d = nc.dram_tensor("x", (C, T, W, H), F32R, kind="ExternalInput").ap()
    wqk_d = nc.dram_tensor("wqk", (128, NTAP, 128), F32R,
                           kind="ExternalInput").ap()
    wv_d = nc.dram_tensor("wv2", (128, NTAP, 128), F32R,
                          kind="ExternalInput").ap()
    gam_d = nc.dram_tensor("gamma_bc", (128, 1), F32,
                           kind="ExternalInput").ap()
    ones_d = nc.dram_tensor("ones", (128, BN), F32R,
                            kind="ExternalInput").ap()
    zer_d = nc.dram_tensor("zer", (128, HP), F32R,
                           kind="ExternalInput").ap()
    out_d = nc.dram_tensor("out", (C, T, W, H), F32,
                           kind="ExternalOutput").ap()
    from contextlib import ExitStack
    with tile.TileContext(nc) as tc, ExitStack() as ctx:
        _emit(nc, tc, x_d, wqk_d, wv_d, gam_d, ones_d, zer_d, out_d, ctx)
    nc.compile()
    _CACHE["nc"] = nc
    return nc


def run_spmd(x, wq, wk, wv, bq, bk, bv, gamma, trace=False, **kw):
    nc = _build()
    wqk, wv2 = _pack_weights(
        np.asarray(wq, np.float32), np.asarray(wk, np.float32),
        np.asarray(wv, np.float32), np.asarray(bq, np.float32),
        np.asarray(bk, np.float32), np.asarray(bv, np.float32))
    gam = np.full((128, 1), np.float32(np.asarray(gamma).reshape(-1)[0]),
                  np.float32)
    x = np.asarray(x, np.float32)
    in_maps = [
        {"x": np.ascontiguousarray(x[b]), "wqk": wqk, "wv2": wv2,
         "gamma_bc": gam, "ones": _ONES, "zer": _ZER}
        for b in range(B)
    ]
    res = bass_utils.run_bass_kernel_spmd(
        nc, in_maps, core_ids=list(range(B)), trace=trace, **kw)
    out = np.stack([res.results[b]["out"] for b in range(B)], axis=0)
    return out, res


def kernel(x, wq, wk, wv, bq, bk, bv, gamma):
    out, _ = run_spmd(x, wq, wk, wv, bq, bk, bv, gamma)
    return out



# revision 2
# speedup vs baseline: 1.0125x; 1.0125x over previous
"""Trainium2 Bass kernel v2 for conv-qkv rank-1 attention.

out = gamma * q * sum((k+bk)*(v+bv)) + x with q,k,v per-time-slice 3x3
convs of x [B=8, C=64, T=16, W=64, H=64]; data-parallel over B on 8 cores.

Key differences vs v1:
- bf16 matmul operands: K=64 bf16 matmuls run ~2x faster per moving row
  than fp32r (measured ~60ns vs 139ns per N=512 matmul).
- Conv bias handling off the PE: k-bias rides the DVE k*v op (op0=add),
  v-bias rides the PSUM->SBUF evacuation, q keeps a 10th ones-tap.
- Contiguous HBM loads + one GpSimd pad-copy (f32->bf16 cast) per pair
  instead of 4096 x 256B strided DMA descriptors per slice.
- Final out = q*s + x fused in one 128-partition DVE pass per block.

Configs (V2_CFG):
- alpha: per block two M=128 chains A1=[q_t|v_t] (rows 0-63), A2=
  [v_t1|q_t1] (rows 64-127) + two M=64 k chains into one swapped bank
  K=[k_t1|k_t]. q evacuated to SBUF, no pipeline lag.
- m64: six M=64 chains per block; Q=[q_t|q_t1] aligned, K/V swapped;
  q conv lags one pair and the final pass reads q straight from PSUM.
"""

import os
from contextlib import ExitStack

import numpy as np
import ml_dtypes

import concourse.bacc as bacc
import concourse.bass as bass
import concourse.mybir as mybir
import concourse.tile as tile
from concourse import bass_utils

F32 = mybir.dt.float32
BF16 = mybir.dt.bfloat16
ALU = mybir.AluOpType

B, C, T, W, H = 8, 64, 16, 64, 64
WP, HP = W + 2, H + 4  # 2-col left pad keeps bf16 copies 4B-aligned
NPAIR = int(os.environ.get("BASS_NPAIR", T // 2))
RB = 8
NBLK = W // RB
BN = RB * H  # 512

CFG = "m64"
BF = ml_dtypes.bfloat16


def _taps(w):  # [O, I, 1, 3, 3] -> [I, 9, O]
    return np.ascontiguousarray(
        np.asarray(w, np.float32).reshape(C, C, 9).transpose(1, 2, 0))


def _pack_weights(wq, wk, wv, bq):
    tq, tk, tv = _taps(wq), _taps(wk), _taps(wv)
    if CFG == "alpha":
        # cols 0:128 = [Wq|Wv] (A1, rows 0-63); 128:256 = [Wv|Wq] (A2,
        # rows 64-127); 256:320 = Wk (both halves). tap 9 = q bias row.
        wp = np.zeros((128, 10, 320), np.float32)
        wp[0:64, 0:9, 0:64] = tq
        wp[0:64, 0:9, 64:128] = tv
        wp[64:128, 0:9, 128:192] = tv
        wp[64:128, 0:9, 192:256] = tq
        wp[0:64, 0:9, 256:320] = tk
        wp[64:128, 0:9, 256:320] = tk
        wp[0, 9, 0:64] = bq
        wp[64, 9, 192:256] = bq
    else:
        # cols 0:64 = Wq (tap 9 = bias), 64:128 = Wk, 128:192 = Wv
        wp = np.zeros((128, 10, 192), np.float32)
        for h in (0, 64):
            wp[h:h + 64, 0:9, 0:64] = tq
            wp[h:h + 64, 0:9, 64:128] = tk
            wp[h:h + 64, 0:9, 128:192] = tv
        wp[0, 9, 0:64] = bq
        wp[64, 9, 0:64] = bq
    return wp.astype(BF)


def _emit(nc, tc, x_d, w_d, biases_d, ones_d, out_d, ctx):
    NCOL = 320 if CFG == "alpha" else 192
    const = ctx.enter_context(tc.tile_pool(name="const", bufs=1))
    state = ctx.enter_context(tc.tile_pool(name="state", bufs=1))
    vpool = ctx.enter_context(tc.tile_pool(name="vpool", bufs=3))
    # alpha uses per-parity tags (A10/A11/A20/A21, K0/K1) with bufs=1:
    # 4 + 2 = 6 PSUM banks; m64 uses single tags with bufs=2: 6 banks.
    nbuf = 1 if CFG == "alpha" else 2
    pA = ctx.enter_context(
        tc.tile_pool(name="pA", bufs=nbuf, space=bass.MemorySpace.PSUM))
    pK = ctx.enter_context(
        tc.tile_pool(name="pK", bufs=nbuf, space=bass.MemorySpace.PSUM))

    w_t = const.tile([128, 10, NCOL], BF16, tag="w")
    ones_t = const.tile([128, RB, H], BF16, tag="ones")
    bias_t = const.tile([128, 3], F32, tag="biases")  # cols: bk, bv, gamma
    nc.sync.dma_start(w_t[:], w_d[:])
    nc.sync.dma_start(ones_t[:], ones_d[:])
    nc.sync.dma_start(bias_t[:], biases_d[:])
    bk2 = bias_t[:, 0:1]
    gam = bias_t[:, 2:3]

    NXP = 4
    xp = [state.tile([128, WP, HP], BF16, tag=f"xp{i}", name=f"xp{i}")
          for i in range(NXP)]
    xc = [state.tile([128, W, H], BF16, tag=f"xc{i}", name=f"xc{i}")
          for i in range(2)]
    qs = [state.tile([128, W, H], F32, tag=f"qs{i}", name=f"qs{i}")
          for i in range(2)]
    ot = [state.tile([128, W, H], BF16, tag=f"ot{i}", name=f"ot{i}")
          for i in range(2)]
    scr = state.tile([128, RB, H], BF16, tag="scr")
    sS = [state.tile([128, NBLK], F32, tag=f"sS{i}", name=f"sS{i}")
          for i in range(2)]
    rS = [state.tile([128, 2], F32, tag=f"rS{i}", name=f"rS{i}")
          for i in range(2)]
    sgam = [state.tile([128, 1], F32, tag=f"sg{i}", name=f"sg{i}")
            for i in range(2)]

    # zero the pad rings once, split across engines to shorten the head
    for i, t_ in enumerate(xp):
        (nc.gpsimd if i % 2 else nc.vector).memset(t_[:, :, :], 0.0)

    def load_xp(p):
        # contiguous bf16 load of host-precast x, then pad-copy into the
        # ringed tile split across three engines (ring stays zero)
        c_ = xc[p % 2]
        nc.sync.dma_start(c_[0:64], x_d[:, 2 * p])
        nc.sync.dma_start(c_[64:128], x_d[:, 2 * p + 1])
        t_ = xp[p % NXP]
        nc.vector.tensor_copy(t_[:, 1:45, 2:2 + H], c_[:, 0:44, :])
        nc.scalar.copy(t_[:, 45:59, 2:2 + H], c_[:, 44:58, :])
        nc.gpsimd.tensor_copy(t_[:, 59:65, 2:2 + H], c_[:, 58:64, :])

    def rhs(t_, half, tap, j):
        if tap == 9:
            return ones_t[64 * half:64 * half + 64]
        dy, dx = tap // 3, tap % 3
        return t_[64 * half:64 * half + 64, j * RB + dy:j * RB + dy + RB,
                  dx + 1:dx + 1 + H]

    def chain_tap(t_, j, half, cols, tap, taps, out_ap, tpos):
        if tap >= taps:
            return
        nc.tensor.matmul(
            out_ap, w_t[64 * half:64 * half + 64, tap, cols[0]:cols[1]],
            rhs(t_, half, tap, j), start=(tap == 0),
            stop=(tap == taps - 1), tile_position=tpos)

    def s_finalize(w, swap):
        r = rS[w % 2]
        nc.vector.reduce_sum(r[:, 0:1], sS[w % 2][:, :],
                             axis=mybir.AxisListType.X)
        if swap:
            # sS halves are [s_t1 | s_t]: swap before scaling by gamma
            nc.scalar.dma_start(r[0:64, 1:2], r[64:128, 0:1])
            nc.scalar.dma_start(r[64:128, 1:2], r[0:64, 0:1])
            nc.vector.tensor_scalar_mul(sgam[w % 2][:, :], r[:, 1:2], gam)
        else:
            nc.vector.tensor_scalar_mul(sgam[w % 2][:, :], r[:, 0:1], gam)

    def fs(w, j, in0):
        # out = q * sgam + x, one 128-partition pass
        nc.vector.scalar_tensor_tensor(
            out=ot[w % 2][:, j * RB:(j + 1) * RB, :], in0=in0,
            scalar=sgam[w % 2][:, 0:1],
            in1=xp[w % NXP][:, 1 + j * RB:1 + (j + 1) * RB, 2:2 + H],
            op0=ALU.mult, op1=ALU.add)

    def kv_stt(w, j, kb, vsb):
        nc.vector.scalar_tensor_tensor(
            out=scr[:], in0=kb[:], scalar=bk2, in1=vsb[:],
            op0=ALU.add, op1=ALU.mult, accum_out=sS[w % 2][:, j:j + 1])

    def store_out(w):
        nc.gpsimd.dma_start(out_d[:, 2 * w], ot[w % 2][0:64])
        nc.gpsimd.dma_start(out_d[:, 2 * w + 1], ot[w % 2][64:128])

    load_xp(0)
    if NPAIR > 1:
        load_xp(1)

    if CFG == "alpha":
        for w in range(NPAIR):
            if w + 2 < NPAIR:
                load_xp(w + 2)
            t_ = xp[w % NXP]
            qs_, ot_ = qs[w % 2], ot[w % 2]
            vsbs = []
            # A phase: M=128 chains [q_t|v_t] and [v_t1|q_t1], interleaved
            # across block pairs for 4 independent accumulation streams
            for j0 in range(0, NBLK, 2):
                banks = []
                for j in (j0, j0 + 1):
                    a1 = pA.tile([128, RB, H], F32, tag=f"A1{j % 2}", name="a1")
                    a2 = pA.tile([128, RB, H], F32, tag=f"A2{j % 2}", name="a2")
                    banks.append((j, a1, a2))
                for tap in range(10):
                    for (j, a1, a2) in banks:
                        chain_tap(t_, j, 0, (0, 128), tap, 10, a1[:], (0, 0))
                        chain_tap(t_, j, 1, (128, 256), tap, 10, a2[:],
                                  (64, 0))
                for (j, a1, a2) in banks:
                    vsb = vpool.tile([128, RB, H], BF16, tag="vsb", name="vsb")
                    jsl = slice(j * RB, (j + 1) * RB)
                    nc.scalar.copy(qs_[0:64, jsl, :], a1[0:64])
                    nc.scalar.copy(qs_[64:128, jsl, :], a2[64:128])
                    nc.scalar.add(vsb[64:128], a1[64:128],
                                  bias_t[64:128, 1:2])
                    nc.vector.tensor_scalar_add(vsb[0:64], a2[0:64],
                                                bias_t[0:64, 1:2])
                    vsbs.append(vsb)
            # K phase: M=64 chains into swapped banks [k_t1 | k_t],
            # 4 half-bank streams across block pairs
            for j0 in range(0, NBLK, 2):
                kbs = [(j, pK.tile([128, RB, H], F32, tag=f"K{j % 2}", name="kb"))
                       for j in (j0, j0 + 1)]
                for tap in range(9):
                    for (j, kb) in kbs:
                        chain_tap(t_, j, 1, (256, 320), tap, 9, kb[0:64],
                                  (64, 0))
                        chain_tap(t_, j, 0, (256, 320), tap, 9, kb[64:128],
                                  (0, 64))
                for (j, kb) in kbs:
                    kv_stt(w, j, kb, vsbs[j])
            s_finalize(w, swap=True)
            for j in range(NBLK):
                fs(w, j, qs_[:, j * RB:(j + 1) * RB, :])
            store_out(w)
    else:
        pQ = ctx.enter_context(
            tc.tile_pool(name="pQ", bufs=4, space=bass.MemorySpace.PSUM))
        for w in range(NPAIR + 1):
            if w + 2 < NPAIR:
                load_xp(w + 2)
            tq_ = xp[(w - 1) % NXP]
            tkv_ = xp[w % NXP]
            # Merged per-block emission, six chains round-robin per tap:
            # [K-lo, K-hi, V-lo, V-hi, Q-lo, Q-hi]. kv for pair w into
            # swapped banks K=[k_t1|k_t], V=[v_t1|v_t]; q for pair w-1
            # (PSUM-direct, consumed by fs once s(w-1) is ready).
            for j in range(NBLK):
                mms = []
                if w < NPAIR:
                    kb = pK.tile([128, RB, H], F32, tag="K", name="kb")
                    vb = pA.tile([128, RB, H], F32, tag="V", name="vb")
                    mms += [
                        (tkv_, 1, (64, 128), 9, kb[0:64], (64, 0)),
                        (tkv_, 0, (64, 128), 9, kb[64:128], (0, 64)),
                        (tkv_, 1, (128, 192), 9, vb[0:64], (64, 0)),
                        (tkv_, 0, (128, 192), 9, vb[64:128], (0, 64)),
                    ]
                if w > 0:
                    qb = pQ.tile([128, RB, H], F32, tag="Q", name="qb")
                    mms += [
                        (tq_, 0, (0, 64), 10, qb[0:64], (0, 0)),
                        (tq_, 1, (0, 64), 10, qb[64:128], (64, 64)),
                    ]
                for tap in range(10):
                    for (t_, half, cols, taps, out_ap, tpos) in mms:
                        chain_tap(t_, j, half, cols, tap, taps, out_ap, tpos)
                if w < NPAIR:
                    vsb = vpool.tile([128, RB, H], BF16, tag="vsb", name="vsb")
                    nc.scalar.add(vsb[:], vb[:], bias_t[:, 1:2])
                    kv_stt(w, j, kb, vsb)
                if w > 0:
                    fs(w - 1, j, qb[:])
            if w < NPAIR:
                s_finalize(w, swap=True)
            if w > 0:
                store_out(w - 1)


_CACHE = {}


def _build():
    if CFG in _CACHE:
        return _CACHE[CFG]
    nc = bacc.Bacc("TRN2", target_bir_lowering=False, debug=False,
                   enable_asserts=False, num_devices=8)
    NCOL = 320 if CFG == "alpha" else 192
    x_d = nc.dram_tensor("x", (C, T, W, H), BF16, kind="ExternalInput").ap()
    w_d = nc.dram_tensor("wpack", (128, 10, NCOL), BF16,
                         kind="ExternalInput").ap()
    biases_d = nc.dram_tensor("biases", (128, 3), F32,
                              kind="ExternalInput").ap()
    ones_d = nc.dram_tensor("ones", (128, RB, H), BF16,
                            kind="ExternalInput").ap()
    out_d = nc.dram_tensor("out", (C, T, W, H), F32,
                           kind="ExternalOutput").ap()
    with tile.TileContext(nc) as tc, ExitStack() as ctx:
        _emit(nc, tc, x_d, w_d, biases_d, ones_d, out_d, ctx)
    nc.compile()
    _CACHE[CFG] = nc
    return nc


_ONES = np.ones((128, RB, H), BF)


def run_spmd(x, wq, wk, wv, bq, bk, bv, gamma, trace=False, **kw):
    nc = _build()
    wp = _pack_weights(wq, wk, wv, np.asarray(bq, np.float32))
    biases = np.zeros((128, 3), np.float32)
    for h in (0, 64):
        biases[h:h + 64, 0] = np.asarray(bk, np.float32)
        biases[h:h + 64, 1] = np.asarray(bv, np.float32)
    biases[:, 2] = np.float32(np.asarray(gamma).reshape(-1)[0])
    x = np.asarray(x, np.float32)
    in_maps = [
        {"x": np.ascontiguousarray(x[b]).astype(BF), "wpack": wp,
         "biases": biases,
         "ones": _ONES}
        for b in range(B)
    ]
    res = bass_utils.run_bass_kernel_spmd(
        nc, in_maps, core_ids=list(range(B)), trace=trace, **kw)
    out = np.stack([res.results[b]["out"] for b in range(B)], axis=0)
    return out, res


def kernel(x, wq, wk, wv, bq, bk, bv, gamma):
    out, _ = run_spmd(x, wq, wk, wv, bq, bk, bv, gamma)
    return out
